# revision 15
# baseline (speedup 1.0000x reference)
"""Trainium2 Bass kernel for nn_LocalCausalGraph — polynomial factorization.

Math: out[b,i,j] = sigmoid(b2 + sum_c W2_c * gelu(u[b,i,c] + v[b,j,c]))
  u = x @ (W1c@Wc).T  (cause path),  v = x @ (W1e@We).T + b1 (effect path)

Key idea: gelu(x) = x/2 + e(x) with e EVEN.  Per channel c, approximate
  e(u+v) ~= sum_{l=0..Lv} A_l(u) * psi_l(v)   on the realized (u,v) box,
  psi_l(v) = vt^{l%2} * wv^{l//2},  vt = v/sv_c, wv = vt^2 - 1/2
  A_l(u)   = ut^{l%2} * poly_{K_l}(wu),  ut = u/su_c, wu = ut^2 - 1/2
so the whole pairwise-grid nonlinearity folds into ONE matmul with
contraction (c, l) — no 67M-element activation at all.  The exact linear
part (u+v)/2 folds into slots l=0,1.  Coefficients are least-squares fit
per channel on the host (they depend only on per-channel input ranges) and
ride in as per-partition scalar operands of the Vector-engine Horner chain.

Sharding: core k owns i-rows [k*64, (k+1)*64) of every batch (needs full
v-projection per batch, which is tiny, plus its own u-slice).

Layout: partitions pair (64 channels) x (2 slots); the Horner chain for
slot-pair tiles runs both parities at once with per-partition coefs, a
final fused (acc + G0) * [1; ut] step applies the constant term and the
odd-parity multiplier together.  The v-side is an NT-tile power chain
P_t = P_{t-1} * (vt^2 - 1/2) with the centering folded into the fused
scalar_tensor_tensor step.  Score accumulates NT matmuls into PSUM per
batch; the tail is a single ScalarE sigmoid straight out of PSUM.
"""

import os
import numpy as np
import ml_dtypes

import concourse.bass as bass
import concourse.bacc as bacc
import concourse.mybir as mybir
import concourse.tile as tile

FP32 = mybir.dt.float32
BF16 = mybir.dt.bfloat16
AF = mybir.ActivationFunctionType
ALU = mybir.AluOpType

B, L, D, CD = 4, 512, 1024, 64
N_CORES = 8
IC = L // N_CORES          # i-rows per core per batch = 64
DT = D // 128              # contraction d-tiles = 8

LV = int(os.environ.get("KLV", "7"))      # max v-basis index (slots = LV+1)
DTOT = int(os.environ.get("KDTOT", "14"))  # total 2D degree cap
NT = (LV + 2) // 2                         # slot pair-tiles = 5
KS = [(DTOT - 2 * t) // 2 for t in range(NT)]   # Horner length per tile
NCOEF = 5 + sum(k + 1 for k in KS) + 3     # coef table columns


def build_kernel(reps: int = 1) -> bass.Bass:
    nc = bacc.Bacc()

    xt = nc.declare_dram_parameter("xt", [B, D, L], BF16, isOutput=False)
    # xti: (128, DT, B, IC) partition-major slice of this core's i-columns
    xti = nc.declare_dram_parameter("xti", [128, DT * B * IC], BF16, isOutput=False)
    # host-combined dup-column projection weights (d-part, DT x [h|h])
    met = nc.declare_dram_parameter("met", [128, DT * 128], BF16, isOutput=False)
    mct = nc.declare_dram_parameter("mct", [128, DT * 128], BF16, isOutput=False)
    coefs = nc.declare_dram_parameter("coefs", [128, NCOEF], FP32, isOutput=False)
    out = nc.declare_dram_parameter("out", [B, IC, L], FP32, isOutput=True)

    import contextlib

    with tile.TileContext(nc) as tc:
        with (
            tc.tile_pool(name="const", bufs=1) as const,
            tc.tile_pool(name="vwork", bufs=2) as vwork,
            tc.tile_pool(name="pp", bufs=2, space="PSUM") as pp,
            tc.tile_pool(name="phcp", bufs=1, space="PSUM") as phcp,
            tc.tile_pool(name="psc", bufs=4, space="PSUM") as psc,
            tc.For_i(0, reps, 1) if reps > 1 else contextlib.nullcontext(),
        ):
            # ---- DMAs in critical-path priority order; coef table rides the
            # scalar-engine ring so its tail WAR can't stall the big queue
            cf = const.tile([128, NCOEF], FP32)
            nc.scalar.dma_start(out=cf, in_=coefs[:, :])
            mct_sb = const.tile([128, DT * 128], BF16)
            nc.sync.dma_start(out=mct_sb, in_=mct[:, :])
            xti_sb = const.tile([128, DT, B * IC], BF16)
            nc.sync.dma_start(
                out=xti_sb.rearrange("p a b -> p (a b)"), in_=xti[:, :]
            )
            met_sb = const.tile([128, DT * 128], BF16)
            nc.sync.dma_start(out=met_sb, in_=met[:, :])
            xt_sb = const.tile([128, B, DT, L], BF16)
            nc.sync.dma_start(
                out=xt_sb[:, 0, 0:DT // 2, :],
                in_=xt[0, 0:D // 2].rearrange("(dt p) l -> p dt l", p=128),
            )
            nc.sync.dma_start(
                out=xt_sb[:, 0, DT // 2:, :],
                in_=xt[0, D // 2:].rearrange("(dt p) l -> p dt l", p=128),
            )
            for b in range(1, B):
                nc.sync.dma_start(
                    out=xt_sb[:, b, :, :],
                    in_=xt[b].rearrange("(dt p) l -> p dt l", p=128),
                )

            # coef table column map (see prep_inputs)
            invsu = cf[:, 0:1]
            b1dup = cf[:, 1:2]
            invsv = cf[:, 2:3]
            maskA = cf[:, 3:4]   # [0;1]
            maskB = cf[:, 4:5]   # [1;0]
            gcol = 5
            gof = {}
            for t in range(NT):
                gof[t] = gcol
                gcol += KS[t] + 1
            fold1 = cf[:, gcol:gcol + 1]          # [W2*su/2 ; 0]
            fold2 = cf[:, gcol + 1:gcol + 2]      # [0 ; W2*sv/2]
            b2c = cf[0:CD, gcol + 2:gcol + 3]     # b2 on partitions 0:64

            # ---- u-side: hc for ALL batches at once -> A feature tiles ----
            hc_ps = phcp.tile([128, B * IC], FP32, tag="phc")
            for ch in range(DT):
                nc.tensor.matmul(
                    hc_ps,
                    lhsT=mct_sb[:, ch * 128:(ch + 1) * 128],
                    rhs=xti_sb[:, ch, :],
                    start=(ch == 0), stop=(ch == DT - 1),
                )
            ut = const.tile([128, B * IC], BF16)
            nc.vector.tensor_scalar_mul(ut, hc_ps, invsu)
            sq = const.tile([128, B * IC], BF16)
            nc.gpsimd.tensor_tensor(sq, ut, ut, ALU.mult)
            wu = const.tile([128, B * IC], BF16)
            nc.vector.tensor_scalar_add(wu, sq, -0.5)
            mt = const.tile([128, B * IC], BF16)   # [1 ; ut]
            nc.vector.tensor_scalar(mt, ut, maskA, maskB, ALU.mult, ALU.add)

            A = []
            for t in range(NT):
                K = KS[t]
                g0 = gof[t]
                acc = vwork.tile([128, B * IC], BF16, tag="acc")
                nc.vector.tensor_scalar_mul(acc, wu, cf[:, g0 + K:g0 + K + 1])
                for k in range(K - 1, 0, -1):
                    acc2 = vwork.tile([128, B * IC], BF16, tag="acc")
                    nc.vector.scalar_tensor_tensor(
                        acc2, acc, cf[:, g0 + k:g0 + k + 1], wu, ALU.add, ALU.mult
                    )
                    acc = acc2
                At = const.tile([128, B * IC], BF16, name=f"A_{t}")
                nc.vector.scalar_tensor_tensor(
                    At, acc, cf[:, g0:g0 + 1], mt, ALU.add, ALU.mult
                )
                A.append(At)
            # exact linear part: A0 += (W2*su/2)*ut (top), A1 += W2*sv/2 (bottom)
            A0f = const.tile([128, B * IC], BF16, name="A0f")
            nc.vector.scalar_tensor_tensor(A0f, ut, fold1, A[0], ALU.mult, ALU.add)
            nc.vector.tensor_scalar(A[0], A0f, fold2, None, ALU.add)
            # (A[0] = A0f + fold2 reuses A[0] tile as final)

            # ---- per-batch: he -> v-feature chain -> score -> sigmoid ----
            out_sb = const.tile([CD, B * L], FP32)
            he_ps = {}

            def he_proj(b):
                he_ps[b] = pp.tile([128, L], FP32, tag="pbig", name=f"he_{b}")
                for ch in range(DT):
                    nc.tensor.matmul(
                        he_ps[b],
                        lhsT=met_sb[:, ch * 128:(ch + 1) * 128],
                        rhs=xt_sb[:, b, ch, :],
                        start=(ch == 0), stop=(ch == DT - 1),
                    )

            he_proj(0)
            for b in range(B):
                if b + 1 < B:
                    he_proj(b + 1)
                vt = vwork.tile([128, L], BF16, tag="vt")
                nc.vector.tensor_scalar(vt, he_ps[b], b1dup, invsv, ALU.add, ALU.mult)
                vsq = vwork.tile([128, L], BF16, tag="vsq")
                nc.gpsimd.tensor_tensor(vsq, vt, vt, ALU.mult)
                P = vwork.tile([128, NT, L], BF16, tag="P")
                nc.vector.tensor_scalar(P[:, 0, :], vt, maskA, maskB, ALU.mult, ALU.add)
                for t in range(1, NT):
                    # P_t = P_{t-1} * (vt^2 - 1/2)
                    nc.vector.scalar_tensor_tensor(
                        P[:, t, :], vsq, -0.5, P[:, t - 1, :], ALU.add, ALU.mult
                    )

                sc = psc.tile([CD, L], FP32, tag="sc", name=f"sc_{b}")
                for t in range(NT):
                    nc.tensor.matmul(
                        sc,
                        lhsT=A[t][:, b * IC:(b + 1) * IC],
                        rhs=P[:, t, :],
                        start=(t == 0), stop=(t == NT - 1),
                    )
                nc.scalar.activation(
                    out_sb[:, b * L:(b + 1) * L], sc, AF.Sigmoid, bias=b2c
                )
                nc.sync.dma_start(out=out[b], in_=out_sb[:, b * L:(b + 1) * L])

    nc.finalize()
    return nc


_erf = np.vectorize(__import__("math").erf)


def _gelu_e(z):
    g = 0.5 * z * (1.0 + _erf(z / np.sqrt(2.0)))
    return g - z / 2


def _fit_channel(su, sv, Lv, Dtot, ngrid=72, margin=1.03):
    ug = np.linspace(-su * margin, su * margin, ngrid)
    vg = np.linspace(-sv * margin, sv * margin, ngrid)
    U, V = np.meshgrid(ug, vg, indexing="ij")
    F = _gelu_e(U + V)
    utg, vtg = U / su, V / sv
    wug, wvg = utg * utg - 0.5, vtg * vtg - 0.5
    cols, idx = [], []
    for l in range(Lv + 1):
        K = (Dtot - l) // 2
        psi = (vtg ** (l % 2)) * (wvg ** (l // 2))
        base = np.ones_like(utg) if l % 2 == 0 else utg
        for k in range(K + 1):
            cols.append((base * wug ** k * psi).ravel())
            idx.append((l, k))
    Amat = np.stack(cols, -1)
    coef, *_ = np.linalg.lstsq(Amat, F.ravel(), rcond=None)
    return idx, coef


def prep_inputs(x, Wc, We, W1, b1, W2, b2):
    bf = ml_dtypes.bfloat16
    xtf = np.ascontiguousarray(x.transpose(0, 2, 1)).astype(bf)   # (B, D, L)

    # combined projection weights, bf16, in dup-column device layout:
    # (128 d-in-chunk, DT chunks x [64 h | 64 h])
    Mcb = (W1[:, :CD] @ Wc).astype(bf).astype(np.float32)   # (CD, D)
    Meb = (W1[:, CD:] @ We).astype(bf).astype(np.float32)

    def dup_layout(M):
        t = M.T.reshape(DT, 128, CD)            # (DT, 128, CD)
        out_ = np.zeros((128, DT, 128), np.float32)
        out_[:, :, 0:CD] = t.transpose(1, 0, 2)
        out_[:, :, CD:128] = t.transpose(1, 0, 2)
        return out_.reshape(128, DT * 128).astype(bf)

    mct_h = dup_layout(Mcb)
    met_h = dup_layout(Meb)

    xb = x.astype(bf).astype(np.float32)
    u = np.einsum("bld,cd->blc", xb, Mcb)
    v = np.einsum("bld,cd->blc", xb, Meb) + b1
    su = np.abs(u).max(axis=(0, 1)) * 1.04 + 1e-6
    sv = np.abs(v).max(axis=(0, 1)) * 1.04 + 1e-6

    W2v = W2[0].astype(np.float32)
    coefs = np.zeros((128, NCOEF), np.float32)
    coefs[0:CD, 0] = 1.0 / su
    coefs[CD:128, 0] = 1.0 / su
    coefs[0:CD, 1] = b1
    coefs[CD:128, 1] = b1
    coefs[0:CD, 2] = 1.0 / sv
    coefs[CD:128, 2] = 1.0 / sv
    coefs[0:CD, 3] = 0.0      # maskA = [0;1]
    coefs[CD:128, 3] = 1.0
    coefs[0:CD, 4] = 1.0      # maskB = [1;0]
    coefs[CD:128, 4] = 0.0
    gcol = 5
    gof = {}
    for t in range(NT):
        gof[t] = gcol
        gcol += KS[t] + 1
    for c in range(CD):
        idx, coef = _fit_channel(su[c], sv[c], LV, DTOT)
        for (l, k), cv in zip(idx, coef):
            t, half = l // 2, l % 2
            coefs[half * CD + c, gof[t] + k] = cv * W2v[c]
    coefs[0:CD, gcol] = W2v * su / 2.0        # fold1 top
    coefs[CD:128, gcol] = 0.0
    coefs[0:CD, gcol + 1] = 0.0               # fold2 bottom
    coefs[CD:128, gcol + 1] = W2v * sv / 2.0
    coefs[0:CD, gcol + 2] = b2[0]             # sigmoid bias

    shared = {"xt": xtf, "met": met_h, "mct": mct_h, "coefs": coefs}
    in_maps = []
    for k in range(N_CORES):
        m = dict(shared)
        sl = xtf[:, :, k * IC:(k + 1) * IC].reshape(B, DT, 128, IC)
        # (128, DT, B, IC)
        m["xti"] = np.ascontiguousarray(
            sl.transpose(2, 1, 0, 3).reshape(128, DT * B * IC)
        )
        in_maps.append(m)
    return in_maps


def kernel(x, Wc, We, W1, b1, W2, b2):
    from concourse.bass_utils import run_bass_kernel_spmd

    x, Wc, We, W1, b1, W2, b2 = (
        np.asarray(a) for a in (x, Wc, We, W1, b1, W2, b2)
    )
    nc = build_kernel()
    in_maps = prep_inputs(x, Wc, We, W1, b1, W2, b2)
    res = run_bass_kernel_spmd(nc, in_maps, list(range(N_CORES)))
    full = np.empty((B, L, L), np.float32)
    for k in range(N_CORES):
        full[:, k * IC:(k + 1) * IC, :] = res.results[k]["out"]
    return full
